# revision 5
# baseline (speedup 1.0000x reference)
"""Causal MHA (B=2,T=2048,C=1024,H=16,Ca=64) on 8 trn2 cores - fp8 DoubleRow v2.

Sharding: core c gets batch b=c//4, heads [4g,4g+4), g=c%4; partial output
projections through 256-row w_o slices summed on host (bf16 partials).

Design (vs the f32r/bf16 chunk-pipelined baseline):
  - QKV projections via fp8e4 DoubleRow matmuls (2 k-tiles per PE pass):
    the host pre-splits x (x4 scale) and w_q/w_k/w_v (x16 scale) into fp8
    hi/lo pairs; 3 cross terms (hi.hi + hi.lo + lo.hi) recover ~f32r
    accuracy at 3/4 of the f32r PE cost. Scales avoid fp8 subnormals; the
    exp scale and w_o absorb them exactly.
  - Scores via fp8 DoubleRow with Ca=64 split 2x32: Q/K drain psum->fp8,
    then DMA through a DRAM scratch into the [32(j),2(i),t] layout (w
    columns host-ordered (hl,j,i) making the scratch writes contiguous).
    Half the bf16 score cost.
  - Each chunk's first two head tasks emit their diagonal score pairs
    first from bf16 raw-drain-layout tiles (no shuffle wait), keeping the
    Act engine's exp stream fed across chunk boundaries while the fp8
    shuffle completes.
  - Triangle masks multiply post-exp on the idle GpSimd engine.
  - AV stays bf16 natural-layout with a ones column emitting softmax row
    sums; AV+normalization run as deferred closures popped as PE fillers
    in the Act-bound second half, with a pb ring for chunks 2-3.
  - All PE work is cut into ~0.6us filler sub-units popped between score
    pairs under a group FIFO with force-drain for cross dependencies.
  - Output projection bf16 (y^T via PE transposes); bf16 output partials.
"""

import math
import sys

import numpy as np

for _p in ("/opt/trn_rl_repo",):
    if _p not in sys.path:
        sys.path.insert(0, _p)

import ml_dtypes
import concourse.bass as bass
from concourse import bacc
import concourse.mybir as mybir
from concourse.bass import ts
from concourse.tile import TileContext
from concourse.bass_utils import run_bass_kernel_spmd
from contextlib import ExitStack

F32 = mybir.dt.float32
BF16 = mybir.dt.bfloat16
F8 = mybir.dt.float8e4
DR = mybir.MatmulPerfMode.DoubleRow
AF = mybir.ActivationFunctionType
NF8 = ml_dtypes.float8_e4m3
NBF = ml_dtypes.bfloat16

B, T, C = 2, 2048, 1024
H, CA = 16, 64
NCORES = 8
HPC = 4          # heads per core
TB = T // 128    # 16 t-blocks
TC = T // 512    # 4 t-chunks
CK = C // 128    # 8 c-tiles
SX, SW = 4.0, 16.0           # host fp8 pre-scales for x and w
SCALE = (1.0 / math.sqrt(CA)) / (SX * SX * SW * SW)  # exp scale absorbs both
TERMS = ((0, 0), (0, 1), (1, 0))  # (x half, w half): hi*hi + hi*lo + lo*hi


def build_nc():
    nc = bacc.Bacc()
    xhl = nc.declare_dram_parameter("xhl", [2, CK, 128, T], F8, isOutput=False)
    wq_d = nc.declare_dram_parameter("wq", [2, 2, 4, 2, 128, 128], F8, isOutput=False)
    wk_d = nc.declare_dram_parameter("wk", [2, 2, 4, 2, 128, 128], F8, isOutput=False)
    wv_d = nc.declare_dram_parameter("wv", [2, 4, 2, 128, 256], F8, isOutput=False)
    wo_d = nc.declare_dram_parameter("wo", [2, 128, C], BF16, isOutput=False)
    mask_d = nc.declare_dram_parameter("mask", [128, 128], BF16, isOutput=False)
    ident_d = nc.declare_dram_parameter("ident", [128, 128], BF16, isOutput=False)
    out = nc.declare_dram_parameter("out", [TB, 128, C], BF16, isOutput=True)

    with TileContext(nc) as tc, ExitStack() as ctx:
        const = ctx.enter_context(tc.tile_pool(name="const", bufs=1))
        persist = ctx.enter_context(tc.tile_pool(name="persist", bufs=1))
        qpool = ctx.enter_context(tc.tile_pool(name="qpool", bufs=3))
        fpool = ctx.enter_context(tc.tile_pool(name="fpool", bufs=2))
        xpool = ctx.enter_context(tc.tile_pool(name="xpool", bufs=4))
        dscr = ctx.enter_context(tc.tile_pool(name="dscr", bufs=2, space="DRAM"))
        pbp = ctx.enter_context(tc.tile_pool(name="pbp", bufs=4))
        rcp = ctx.enter_context(tc.tile_pool(name="rcp", bufs=2))
        otp = ctx.enter_context(tc.tile_pool(name="otp", bufs=2))
        ps_s = ctx.enter_context(tc.tile_pool(name="ps_s", bufs=2, space="PSUM"))
        po = ctx.enter_context(tc.tile_pool(name="po", bufs=4, space="PSUM"))

        # ---- constants / weights: startup-latency ordered ----
        wq_sb = const.tile([128, 2, 2, 4, 2, 128], F8, tag="wq")
        wk_sb = const.tile([128, 2, 2, 4, 2, 128], F8, tag="wk")
        # first-use order: wq-p0-hi, wk-p0-hi, then x chunk 0, then the rest
        nc.sync.dma_start(wq_sb[:, 0, 0], wq_d[0, 0].rearrange("j i c a -> c j i a"))
        nc.scalar.dma_start(wk_sb[:, 0, 0], wk_d[0, 0].rearrange("j i c a -> c j i a"))
        wv_sb = const.tile([128, 2, 4, 2, 256], F8, tag="wv")
        mask_sb = const.tile([128, 128], BF16, tag="mask")
        nc.scalar.dma_start(mask_sb[:], mask_d[:])
        ident_sb = const.tile([128, 128], BF16, tag="ident")
        wo_sb = const.tile([128, 2, C], BF16, tag="wo")

        dly = persist.tile([1, 4], F8, tag="dly")

        def load_late_consts():
            # Pool-queue DMAs, held back by a 1-element copy that reads the
            # xs3 tile: Pool program order then defers the transfers past
            # the x streams (v units / o-proj consume these much later)
            nc.gpsimd.tensor_copy(dly[:, 0:1], xs_tiles[3][0:1, 0:1, 0:1])
            nc.gpsimd.dma_start(wv_sb[:],
                                wv_d[:].rearrange("w j i c a -> c w j i a"))
            nc.gpsimd.dma_start(ident_sb[:], ident_d[:])
            nc.gpsimd.dma_start(wo_sb[:], wo_d[:].rearrange("c r f -> r c f"))

        # ---- persistent state ----
        k_dr = persist.tile([128, 2, T], F8, tag="kdr")   # [(h j), i, t]
        v_sb = persist.tile([128, HPC, TB, 65], BF16, tag="v")
        nc.vector.memset(v_sb[:, :, :, 64:65], 1.0)
        y_sbT = persist.tile([128, 2, T], BF16, tag="yT")
        y_norm = persist.tile([128, TC, 4, 256], BF16, tag="ynorm")
        # per-chunk bf16 q/k rings in raw drain layout (skip the DRAM
        # shuffle for each chunk's first task; any consistent a-permutation
        # works for S). bufs=2: chunk n+1 may start before (n,0) finishes.
        bfp = ctx.enter_context(tc.tile_pool(name="bfp", bufs=2))
        qbf_tiles = [None] * TC
        kbf_tiles = [None] * TC

        # ---- x chunks: [128, 16, 512] fp8 (hi tiles 0-7, lo 8-15) ----
        xs_tiles = [None] * TC
        xs_tiles[0] = xpool.tile([128, 16, 512], F8, tag="xs", name="xs0")
        nc.sync.dma_start(
            xs_tiles[0][:, 0:4, :],
            xhl[0, 0:4, :, 0:512].rearrange("c r t -> r c t"))
        nc.sync.dma_start(
            xs_tiles[0][:, 4:8, :],
            xhl[0, 4:8, :, 0:512].rearrange("c r t -> r c t"))
        nc.sync.dma_start(
            wq_sb[:, 0, 1], wq_d[0, 1].rearrange("j i c a -> c j i a"))
        nc.scalar.dma_start(
            wk_sb[:, 0, 1], wk_d[0, 1].rearrange("j i c a -> c j i a"))
        nc.sync.dma_start(
            xs_tiles[0][:, 8:16, :],
            xhl[1, :, :, 0:512].rearrange("c r t -> r c t"))
        nc.sync.dma_start(
            wq_sb[:, 1], wq_d[1].rearrange("w j i c a -> c w j i a"))
        nc.sync.dma_start(
            wk_sb[:, 1], wk_d[1].rearrange("w j i c a -> c w j i a"))

        def load_xs(tcn):
            xs_tiles[tcn] = xpool.tile([128, 16, 512], F8, tag="xs",
                                       name=f"xs{tcn}")
            nc.sync.dma_start(
                xs_tiles[tcn][:],
                xhl[:, :, :, ts(tcn, 512)].rearrange("h c r t -> r (h c) t"),
            )

        q_tiles = [None] * TC

        def qkv_units(tcn, cp_eng=None):
            """PE filler units producing Q/K (fp8 DR layout) + V for chunk tcn.

            Returns (qk_units, v_units). qk units include the drain + the
            DRAM-shuffle DMAs; the final unit issues the q_dr/k_dr reads.
            Chunk 0 drains q/k to bf16 raw-layout tiles instead (no q
            shuffle; k still shuffles into k_dr for later chunks).
            """
            xs = xs_tiles[tcn]
            if cp_eng is None:
                cp_eng = nc.vector
            if tcn > 0:
                q_f8 = fpool.tile([128, 2, 512], F8, tag="qf8",
                                  name=f"qf8_{tcn}")
                q_scr = dscr.tile([2, 2, 32, 2, 512], F8, tag="qscr",
                                  name=f"qscr{tcn}")
                q_tiles[tcn] = qpool.tile([128, 2, 512], F8, tag="q",
                                          name=f"q{tcn}")
            else:
                q_f8 = q_scr = None
            k_f8 = fpool.tile([128, 2, 512], F8, tag="kf8", name=f"kf8_{tcn}")
            k_scr = dscr.tile([2, 2, 32, 2, 512], F8, tag="kscr",
                              name=f"kscr{tcn}")
            qbf_tiles[tcn] = bfp.tile([128, 2, 512], BF16, tag="qbf",
                                      name=f"qbf{tcn}")
            kbf_tiles[tcn] = bfp.tile([128, 2, 512], BF16, tag="kbf",
                                      name=f"kbf{tcn}")

            def qk_unit(p, w_sb, f8t, scr, is_q):
                """Returns two ~0.64us sub-units: [matmuls 1-12], [matmuls
                13-24 + drains + shuffle DMAs]."""
                state = {}
                mms = [(xh, wh, j, th) for (xh, wh) in TERMS
                       for j in range(4) for th in range(2)]

                def emit(sel, first):
                    for n, (xh, wh, j, th) in enumerate(sel):
                        nc.tensor.matmul(
                            state["t"][:, ts(th, 256)],
                            lhsT=w_sb[:, p, wh, j],
                            rhs=xs[:, 8 * xh + 2 * j: 8 * xh + 2 * j + 2,
                                   ts(th, 256)],
                            start=first and n == 0,
                            stop=(xh, wh) == TERMS[-1] and j == 3 and th == 1,
                            perf_mode=DR,
                            skip_group_check=True,
                        )

                def u_a():
                    state["t"] = po.tile([128, 512], F32, tag="po",
                                         name="po_qk")
                    emit(mms[:12], True)

                def u_b():
                    emit(mms[12:], False)
                    t = state["t"]
                    if tcn == 0 and is_q:
                        # chunk-0 q: bf16 raw-layout only (no shuffle);
                        # scalar drain - it gates the first diag exps, and
                        # Act is idle-waiting on them anyway
                        nc.scalar.copy(qbf_tiles[tcn][:, p, :], t[:])
                        return
                    if tcn == 0 or p == 0:
                        # bf16 raw-layout drain (gates the diag-first exps):
                        # on Act while it still has idle pockets (chunks 0-1),
                        # on DVE once the exp stream saturates
                        beng = nc.scalar if tcn < 2 else nc.vector
                        (beng.copy if beng is nc.scalar
                         else beng.tensor_copy)(
                            qbf_tiles[tcn][:, p, :] if is_q
                            else kbf_tiles[tcn][:, p, :], t[:])
                    nc.vector.tensor_copy(f8t[:, p, :], t[:])
                    # contiguous scratch write for this p
                    nc.sync.dma_start(
                        scr[p].rearrange("hl j i t -> (hl j i) t"),
                        f8t[:, p, :])
                    if p == 1:
                        # both halves written: read back in DR layout
                        if is_q:
                            nc.sync.dma_start(
                                q_tiles[tcn][:],
                                scr[:].rearrange("p hl j i t -> (p hl j) i t"))
                        else:
                            nc.sync.dma_start(
                                k_dr[:, :, ts(tcn, 512)],
                                scr[:].rearrange("p hl j i t -> (p hl j) i t"))
                return [u_a, u_b]

            def v_unit(tbl, eng):
                def u():
                    t = po.tile([128, 256], F32, tag="po", name="po_v")
                    first = True
                    for (xh, wh) in TERMS:
                        for j in range(4):
                            nc.tensor.matmul(
                                t[:],
                                lhsT=xs[:, 8 * xh + 2 * j: 8 * xh + 2 * j + 2,
                                        ts(tbl, 128)],
                                rhs=wv_sb[:, wh, j],
                                start=first,
                                stop=(xh, wh) == TERMS[-1] and j == 3,
                                perf_mode=DR,
                                skip_group_check=True,
                            )
                            first = False
                    cp = eng.copy if eng is nc.scalar else eng.tensor_copy
                    cp(
                        v_sb[:, :, 4 * tcn + tbl, 0:64],
                        t[:].rearrange("r (h a) -> r h a", h=HPC),
                    )
                return u

            qk_a = (qk_unit(0, wq_sb, q_f8, q_scr, True)
                    + qk_unit(0, wk_sb, k_f8, k_scr, False))
            qk_b = (qk_unit(1, wq_sb, q_f8, q_scr, True)
                    + qk_unit(1, wk_sb, k_f8, k_scr, False))
            veng = nc.vector if tcn >= 2 else cp_eng
            vs = [v_unit(tbl, veng) for tbl in range(4)]
            return qk_a, qk_b, vs

        def tr_unit(tcn, tbl, cjs, eng=None):
            """PE-transpose y_norm(tcn, tbl) column block(s) cj into y_sbT."""
            def u():
                tb = 4 * tcn + tbl
                t = po.tile([128, len(cjs), 128], BF16, tag="po", name="po_tr")
                for i, cj in enumerate(cjs):
                    nc.tensor.transpose(
                        t[:, i, :],
                        y_norm[:, tcn, tbl, ts(cj, 128)],
                        ident_sb[:],
                    )
                e = eng or nc.vector
                cp = e.copy if e is nc.scalar else e.tensor_copy
                if len(cjs) == 2:
                    cp(y_sbT[:, :, ts(tb, 128)], t[:])
                else:
                    cp(y_sbT[:, cjs[0], ts(tb, 128)], t[:, 0, :])
            return u

        def o_unit(tcn, tbl, eng=None):
            """Two sub-units (one per 512-wide half of the projection)."""
            tb = 4 * tcn + tbl
            e = eng or nc.vector
            cp = e.copy if e is nc.scalar else e.tensor_copy
            state = {}

            def half(cc):
                t = po.tile([128, 512], F32, tag="po", name="po_o")
                for cj in range(2):
                    nc.tensor.matmul(
                        t[:], lhsT=y_sbT[:, cj, ts(tb, 128)],
                        rhs=wo_sb[:, cj, ts(cc, 512)],
                        start=(cj == 0), stop=(cj == 1),
                    )
                cp(state["ot"][:, ts(cc, 512)], t[:])

            def u_a():
                state["ot"] = otp.tile([128, C], BF16, tag="ot", name="ot")
                half(0)

            def u_b():
                half(1)
                nc.sync.dma_start(out[tb][:], state["ot"][:])
            return [u_a, u_b]

        def out_units(tcn):
            units = [tr_unit(tcn, tbl, (0, 1)) for tbl in range(4)]
            for tbl in range(4):
                units += o_unit(tcn, tbl)
            return units

        # ---------------- global head-task pipeline ----------------
        # Fillers are (group, fn) pairs popped FIFO between score pairs.
        # Insertion order = dependency order; drain_fillers() force-pops a
        # group (and everything queued ahead of it) before dependent work.
        fillers = []
        av_closures = {0: [], 1: [], 2: [], 3: []}

        def add_fillers(group, units):
            fillers.extend((group, u) for u in units)

        def pop_filler():
            if fillers:
                fillers.pop(0)[1]()

        def drain_fillers(group):
            while any(g == group for g, _ in fillers):
                fillers.pop(0)[1]()

        def attn_task(tcn, h, pops=(), bf16=False, diag_first=False,
                      ring_pb=False, drain_at=None, inline_av=False):
            q_ap = q_tiles[tcn]
            npairs = 2 * tcn + 2
            pb_tiles = {}
            p_, hl_ = h // 2, h % 2
            yt = (po.tile([128, 4, 65], F32, tag="po", name="ypsi")
                  if inline_av else None)

            def sc_mm(sps, col, sb, t0, w, first, last, use_bf):
                """One score matmul: s-block sb, t range [t0, t0+w)."""
                if use_bf:
                    # bf tiles hold only this chunk: local s-block index
                    nc.tensor.matmul(
                        sps[:, col:col + w],
                        lhsT=kbf_tiles[tcn][64 * hl_:64 * hl_ + 64, p_,
                                            ts(sb - 4 * tcn, 128)],
                        rhs=qbf_tiles[tcn][64 * hl_:64 * hl_ + 64, p_,
                                           t0:t0 + w],
                        start=first, stop=last,
                        skip_group_check=True,
                    )
                    return
                nc.tensor.matmul(
                    sps[:, col:col + w],
                    lhsT=k_dr[32 * h:32 * h + 32, :, ts(sb, 128)],
                    rhs=q_ap[32 * h:32 * h + 32, :, t0:t0 + w],
                    start=first, stop=last,
                    perf_mode=DR,
                    skip_group_check=True,
                    tile_position=(32 * h, 0),
                )

            def av_emit(pi, yt, arm, finish):
                pb = pb_tiles[pi]
                if pi < 2 * tcn:
                    sbs = [(2 * pi, lambda tbl: 128 * tbl, 0),
                           (2 * pi + 1, lambda tbl: 512 + 128 * tbl, 0)]
                elif pi == 2 * tcn:
                    sbs = [(4 * tcn, lambda tbl: 128 * tbl, 0),
                           (4 * tcn + 1, lambda tbl: 384 + 128 * tbl, 1)]
                else:
                    sbs = [(4 * tcn + 2, lambda tbl: 128 * tbl - 256, 2),
                           (4 * tcn + 3, lambda tbl: 256, 3)]
                mms = [(sb, colf(tbl), tbl)
                       for sb, colf, tbl0 in sbs for tbl in range(tbl0, 4)]
                for i, (sb, c0, tbl) in enumerate(mms):
                    nc.tensor.matmul(
                        yt[:, tbl, :],
                        lhsT=pb[:, c0:c0 + 128],
                        rhs=v_sb[:, h, sb, :],
                        start=(arm and i == 0),
                        stop=(finish and i == len(mms) - 1),
                        skip_group_check=True,
                    )

            def emit_pair(pi, use_bf):
                if pi == 2 * tcn + 1 and tcn < 3:
                    sps = po.tile([128, 512], F32, tag="po", name="spsb")
                else:
                    sps = ps_s.tile([128, 1024], F32, tag="s", name="sps")
                if ring_pb:
                    pb = pbp.tile([128, 1024], BF16, name="pb", bufs=26,
                                  tag="pb")
                else:
                    pb = pbp.tile([128, 1024], BF16, name="pbd", bufs=1,
                                  tag=f"pbd{tcn}{h}{pi}")
                pb_tiles[pi] = pb
                if pi < 2 * tcn:
                    # two full 512-wide s-blocks; banks at cols 0/512
                    for jj in range(2):
                        sb = 2 * pi + jj
                        sc_mm(sps, 512 * jj, sb, 0, 256, True, False, use_bf)
                        sc_mm(sps, 512 * jj + 256, sb, 256, 256, False, True,
                              use_bf)
                    nc.scalar.activation(pb[:], sps[:], AF.Exp, scale=SCALE)
                elif pi == 2 * tcn:
                    # diagonal pair A: d0 (512 wide) + d1 (384 wide);
                    # triangle masks multiply on the (idle) Pool engine
                    sc_mm(sps, 0, 4 * tcn, 0, 256, True, False, use_bf)
                    sc_mm(sps, 256, 4 * tcn, 256, 256, False, True, use_bf)
                    sc_mm(sps, 512, 4 * tcn + 1, 128, 256, True, False, use_bf)
                    sc_mm(sps, 768, 4 * tcn + 1, 384, 128, False, True,
                          use_bf)
                    nc.scalar.activation(pb[:, 0:896], sps[:, 0:896],
                                         AF.Exp, scale=SCALE)
                    nc.gpsimd.tensor_mul(pb[:, 0:128], pb[:, 0:128],
                                         mask_sb[:])
                    nc.gpsimd.tensor_mul(pb[:, 512:640], pb[:, 512:640],
                                         mask_sb[:])
                else:
                    # diagonal pair B: d2 (256 wide) + d3 (128 wide)
                    sc_mm(sps, 0, 4 * tcn + 2, 256, 256, True, False, use_bf)
                    sc_mm(sps, 256, 4 * tcn + 3, 384, 128, False, True,
                          use_bf)
                    nc.scalar.activation(pb[:, 0:384], sps[:, 0:384],
                                         AF.Exp, scale=SCALE)
                    nc.gpsimd.tensor_mul(pb[:, 0:128], pb[:, 0:128],
                                         mask_sb[:])
                    nc.gpsimd.tensor_mul(pb[:, 256:384], pb[:, 256:384],
                                         mask_sb[:])

            if diag_first:
                order = [2 * tcn, 2 * tcn + 1] + list(range(2 * tcn))
            else:
                order = list(range(npairs))
            qka_drained = qkb_drained = False
            for ei, pi in enumerate(order):
                use_bf = bf16 or (diag_first and pi >= 2 * tcn)
                if use_bf and not qka_drained:
                    drain_fillers(f"qka{tcn}")
                    qka_drained = True
                if not use_bf and not qkb_drained:
                    drain_fillers(f"qkb{tcn}")
                    qkb_drained = True
                emit_pair(pi, use_bf)
                if inline_av and ei >= 2:
                    av_emit(order[ei - 2], yt, arm=(ei == 2), finish=False)
                if drain_at and ei in drain_at:
                    drain_fillers(drain_at[ei])
                for _ in range(pops.count(ei)):
                    pop_filler()

            def norm_emit(yt):
                rc = rcp.tile([128, 4], F32, tag="rc", name="rc")
                nc.vector.reciprocal(rc[:], yt[:, :, 64])
                nc.vector.tensor_mul(
                    y_norm[:, tcn, :, 64 * h:64 * h + 64],
                    yt[:, :, 0:64],
                    rc[:].unsqueeze(2).broadcast_to([128, 4, 64]),
                )

            if inline_av:
                av_emit(order[npairs - 2], yt, arm=False, finish=False)
                av_emit(order[npairs - 1], yt, arm=False, finish=True)
                norm_emit(yt)
            else:
                def av_closure():
                    for c in range(tcn + 1):
                        drain_fillers(f"v{c}")
                    yt2 = po.tile([128, 4, 65], F32, tag="po", name="ypsd")
                    for pi in range(npairs):
                        av_emit(pi, yt2, arm=(pi == 0),
                                finish=(pi == npairs - 1))
                    norm_emit(yt2)
                av_closures[tcn].append(av_closure)

        # ---- schedule ----
        qk0a, qk0b, v0 = qkv_units(0)
        for u in qk0a:
            u()
        attn_task(0, 0, bf16=True)
        qk0b[0]()
        qk0b[1]()
        attn_task(0, 1, bf16=True)
        qk0b[2]()
        qk0b[3]()
        load_xs(1)
        qk1a, qk1b, v1 = qkv_units(1)
        add_fillers("qka1", qk1a)
        add_fillers("qkb1", qk1b)
        load_xs(2)
        attn_task(0, 2, pops=(0, 0, 0, 1, 1, 1), bf16=True)
        qk2a, qk2b, v2 = qkv_units(2)
        add_fillers("qka2", qk2a)
        add_fillers("qkb2", qk2b)
        attn_task(0, 3, pops=(0, 0, 0, 1, 1, 1), bf16=True)
        load_xs(3)
        load_late_consts()
        qk3a, qk3b, v3 = qkv_units(3)
        attn_task(1, 0, pops=(0, 1, 2, 3), diag_first=True)
        add_fillers("qka3", qk3a)
        add_fillers("qkb3", qk3b)
        attn_task(1, 1, pops=(0, 1, 2, 3), diag_first=True)
        attn_task(1, 2, pops=(0, 1, 2, 3))
        attn_task(1, 3, pops=(0, 1, 2, 3))
        add_fillers("v0", v0)
        add_fillers("v1", v1)
        add_fillers("v2", v2)
        add_fillers("v3", v3)
        add_fillers("av0", av_closures[0])
        add_fillers("av1", av_closures[1])
        attn_task(2, 0, pops=(0, 0, 1, 1, 2, 2, 3, 4), diag_first=True,
                  ring_pb=True)
        add_fillers("cl20", av_closures[2][0:1])
        attn_task(2, 1, pops=(0, 0, 1, 1, 2, 3, 4, 5), ring_pb=True,
                  diag_first=True)
        add_fillers("cl21", av_closures[2][1:2])
        add_fillers("out0", out_units(0))
        attn_task(2, 2, pops=(0, 0, 1, 1, 2, 3, 4, 5), ring_pb=True)
        add_fillers("cl22", av_closures[2][2:3])
        attn_task(3, 0, pops=(0, 1, 2, 3, 4, 5, 6, 7), diag_first=True,
                  ring_pb=True)
        add_fillers("cl30", av_closures[3][0:1])
        add_fillers("out1", out_units(1))
        attn_task(2, 3, pops=(0, 0, 1, 1, 2, 3, 4, 5), ring_pb=True)
        add_fillers("cl23", av_closures[2][3:4])
        add_fillers("out2", out_units(2))
        attn_task(3, 1, pops=(0, 1, 2, 3, 4, 5, 6, 7), ring_pb=True,
                  diag_first=True)
        add_fillers("cl31", av_closures[3][1:2])
        add_fillers("tr3", [tr_unit(3, tbl, (0,)) for tbl in range(4)])
        attn_task(3, 2, pops=(0, 1, 2, 3, 4, 5, 6, 7), ring_pb=True)
        add_fillers("cl32", av_closures[3][2:3])
        attn_task(3, 3, pops=(0, 1, 2, 3, 4, 5, 6, 7), ring_pb=True)
        add_fillers("cl33", av_closures[3][3:4])
        while fillers:
            pop_filler()
        # chunk-3 tail: all second-half transposes first (drains alternate
        # DVE/Act), then the output projections, so the blocks pipeline
        t1 = [tr_unit(3, tbl, (1,), eng=(nc.scalar if tbl % 2 else None))
              for tbl in range(4)]
        for u in t1:
            u()
        for tbl in range(4):
            for u in o_unit(3, tbl, eng=(nc.scalar if tbl % 2 else None)):
                u()

    nc.compile()
    return nc


_NC = None


def _get_nc():
    global _NC
    if _NC is None:
        _NC = build_nc()
    return _NC


def _mask_arr():
    p = np.arange(128)[:, None]
    f = np.arange(128)[None, :]
    return np.ascontiguousarray((p <= f).astype(NBF))


def _hilo(a, s):
    hi = (s * a).astype(NF8)
    lo = ((s * a) - hi.astype(np.float32)).astype(NF8)
    return hi, lo


def _pack_qk(w, hs):
    """w[H,C,Ca] -> [2(whalf), 2(pair... ) ...] per-core DR pack.

    Layout [w, j, i, c(128), a(128)] per p, with a-columns ordered
    (hl, jj, ii): column m = 64*hl + 2*jj + ii -> head hs[2p+hl] channel
    a = 32*ii + jj.
    """
    out = np.zeros((2, 2, 4, 2, 128, 128), NF8)  # [p, w, pair, tile, c, a]
    m = np.arange(128)
    hl, r = m // 64, m % 64
    jj, ii = r // 2, r % 2
    a_of_m = 32 * ii + jj
    for p in range(2):
        cols = np.stack([w[hs[2 * p + hl_], :, a_] for hl_, a_ in
                         zip(hl, a_of_m)], axis=1)  # [C, 128]
        hi, lo = _hilo(cols, SW)
        for wh, arr in ((0, hi), (1, lo)):
            out[p, wh] = arr.reshape(4, 2, 128, 128)
    return np.ascontiguousarray(out)


def make_in_maps(x, w_q, w_k, w_v, w_o):
    x = np.asarray(x, dtype=np.float32)
    w_q = np.asarray(w_q, dtype=np.float32)
    w_k = np.asarray(w_k, dtype=np.float32)
    w_v = np.asarray(w_v, dtype=np.float32)
    w_o = np.asarray(w_o, dtype=np.float32)
    in_maps = []
    xhl_b = []
    for b in range(B):
        xT = np.ascontiguousarray(x[b].T)                    # [C, T]
        xhi, xlo = _hilo(xT, SX)
        xhl_b.append(np.ascontiguousarray(
            np.stack([xhi, xlo]).reshape(2, CK, 128, T)))
    for c in range(NCORES):
        b, g = c // 4, c % 4
        hs = [4 * g + i for i in range(HPC)]
        wv_cols = np.concatenate([w_v[h] for h in hs], axis=1)  # [C, 256]
        wv_hi, wv_lo = _hilo(wv_cols, SW)
        wv_a = np.stack([wv_hi.reshape(4, 2, 128, 256),
                         wv_lo.reshape(4, 2, 128, 256)])
        wo_a = (w_o[256 * g:256 * (g + 1)] / (SX * SW)).reshape(
            2, 128, C).astype(NBF)
        in_maps.append(dict(
            mask=_mask_arr(),
            ident=np.ascontiguousarray(np.eye(128).astype(NBF)),
            xhl=xhl_b[b],
            wq=_pack_qk(w_q, hs),
            wk=_pack_qk(w_k, hs),
            wv=np.ascontiguousarray(wv_a),
            wo=np.ascontiguousarray(wo_a),
        ))
    return in_maps


def gather_out(results):
    acc = [np.zeros((T, C), np.float64) for _ in range(B)]
    for c in range(NCORES):
        acc[c // 4] += results[c]["out"].reshape(T, C).astype(np.float64)
    return np.stack([a.astype(np.float32) for a in acc])


def run(x, w_q, w_k, w_v, w_o, trace=False, **spmd_kwargs):
    nc = _get_nc()
    in_maps = make_in_maps(x, w_q, w_k, w_v, w_o)
    res = run_bass_kernel_spmd(nc, in_maps, list(range(NCORES)), trace=trace,
                               **spmd_kwargs)
    return gather_out(res.results), res


def kernel(x, w_q, w_k, w_v, w_o):
    out, _ = run(x, w_q, w_k, w_v, w_o)
    return out


# revision 7
# speedup vs baseline: 1.0005x; 1.0005x over previous
"""Causal MHA (B=2,T=2048,C=1024,H=16,Ca=64) on 8 trn2 cores - fp8 DoubleRow v2.

Sharding: core c gets batch b=c//4, heads [4g,4g+4), g=c%4; partial output
projections through 256-row w_o slices summed on host (bf16 partials).

Design (vs the f32r/bf16 chunk-pipelined baseline):
  - QKV projections via fp8e4 DoubleRow matmuls (2 k-tiles per PE pass):
    the host pre-splits x (x4 scale) and w_q/w_k/w_v (x16 scale) into fp8
    hi/lo pairs; 3 cross terms (hi.hi + hi.lo + lo.hi) recover ~f32r
    accuracy at 3/4 of the f32r PE cost. Scales avoid fp8 subnormals; the
    exp scale and w_o absorb them exactly.
  - Scores via fp8 DoubleRow with Ca=64 split 2x32: Q/K drain psum->fp8,
    then DMA through a DRAM scratch into the [32(j),2(i),t] layout (w
    columns host-ordered (hl,j,i) making the scratch writes contiguous).
    Half the bf16 score cost.
  - Each chunk's first two head tasks emit their diagonal score pairs
    first from bf16 raw-drain-layout tiles (no shuffle wait), keeping the
    Act engine's exp stream fed across chunk boundaries while the fp8
    shuffle completes.
  - Triangle masks multiply post-exp on the idle GpSimd engine.
  - AV stays bf16 natural-layout with a ones column emitting softmax row
    sums; AV+normalization run as deferred closures popped as PE fillers
    in the Act-bound second half, with a pb ring for chunks 2-3.
  - All PE work is cut into ~0.6us filler sub-units popped between score
    pairs under a group FIFO with force-drain for cross dependencies.
  - Output projection bf16 (y^T via PE transposes); bf16 output partials.
"""

import math
import sys

import numpy as np

for _p in ("/opt/trn_rl_repo",):
    if _p not in sys.path:
        sys.path.insert(0, _p)

import ml_dtypes
import concourse.bass as bass
from concourse import bacc
import concourse.mybir as mybir
from concourse.bass import ts
from concourse.tile import TileContext
from concourse.bass_utils import run_bass_kernel_spmd
from contextlib import ExitStack

F32 = mybir.dt.float32
BF16 = mybir.dt.bfloat16
F8 = mybir.dt.float8e4
DR = mybir.MatmulPerfMode.DoubleRow
AF = mybir.ActivationFunctionType
NF8 = ml_dtypes.float8_e4m3
NBF = ml_dtypes.bfloat16

B, T, C = 2, 2048, 1024
H, CA = 16, 64
NCORES = 8
HPC = 4          # heads per core
TB = T // 128    # 16 t-blocks
TC = T // 512    # 4 t-chunks
CK = C // 128    # 8 c-tiles
SX, SW = 4.0, 16.0           # host fp8 pre-scales for x and w
SCALE = (1.0 / math.sqrt(CA)) / (SX * SX * SW * SW)  # exp scale absorbs both
TERMS = ((0, 0), (0, 1), (1, 0))  # (x half, w half): hi*hi + hi*lo + lo*hi


def build_nc():
    nc = bacc.Bacc()
    xhl = nc.declare_dram_parameter("xhl", [2, CK, 128, T], F8, isOutput=False)
    wq_d = nc.declare_dram_parameter("wq", [2, 2, 4, 2, 128, 128], F8, isOutput=False)
    wk_d = nc.declare_dram_parameter("wk", [2, 2, 4, 2, 128, 128], F8, isOutput=False)
    wv_d = nc.declare_dram_parameter("wv", [2, 4, 2, 128, 256], F8, isOutput=False)
    wo_d = nc.declare_dram_parameter("wo", [2, 128, C], BF16, isOutput=False)
    mask_d = nc.declare_dram_parameter("mask", [128, 128], BF16, isOutput=False)
    ident_d = nc.declare_dram_parameter("ident", [128, 128], BF16, isOutput=False)
    out = nc.declare_dram_parameter("out", [TB, 128, C], BF16, isOutput=True)

    with TileContext(nc) as tc, ExitStack() as ctx:
        const = ctx.enter_context(tc.tile_pool(name="const", bufs=1))
        persist = ctx.enter_context(tc.tile_pool(name="persist", bufs=1))
        qpool = ctx.enter_context(tc.tile_pool(name="qpool", bufs=3))
        fpool = ctx.enter_context(tc.tile_pool(name="fpool", bufs=2))
        xpool = ctx.enter_context(tc.tile_pool(name="xpool", bufs=4))
        dscr = ctx.enter_context(tc.tile_pool(name="dscr", bufs=2, space="DRAM"))
        pbp = ctx.enter_context(tc.tile_pool(name="pbp", bufs=4))
        rcp = ctx.enter_context(tc.tile_pool(name="rcp", bufs=2))
        otp = ctx.enter_context(tc.tile_pool(name="otp", bufs=2))
        ps_s = ctx.enter_context(tc.tile_pool(name="ps_s", bufs=2, space="PSUM"))
        po = ctx.enter_context(tc.tile_pool(name="po", bufs=4, space="PSUM"))

        # ---- constants / weights: startup-latency ordered ----
        wq_sb = const.tile([128, 2, 2, 4, 2, 128], F8, tag="wq")
        wk_sb = const.tile([128, 2, 2, 4, 2, 128], F8, tag="wk")
        # first-use order: wq-p0-hi, wk-p0-hi, then x chunk 0, then the rest
        nc.sync.dma_start(wq_sb[:, 0, 0], wq_d[0, 0].rearrange("j i c a -> c j i a"))
        nc.scalar.dma_start(wk_sb[:, 0, 0], wk_d[0, 0].rearrange("j i c a -> c j i a"))
        wv_sb = const.tile([128, 2, 4, 2, 256], F8, tag="wv")
        mask_sb = const.tile([128, 128], BF16, tag="mask")
        nc.scalar.dma_start(mask_sb[:], mask_d[:])
        ident_sb = const.tile([128, 128], BF16, tag="ident")
        wo_sb = const.tile([128, 2, C], BF16, tag="wo")

        dly = persist.tile([1, 4], F8, tag="dly")

        def load_late_consts():
            # Pool-queue DMAs, held back by a 1-element copy that reads the
            # xs3 tile: Pool program order then defers the transfers past
            # the x streams (v units / o-proj consume these much later)
            nc.gpsimd.tensor_copy(dly[:, 0:1], xs_tiles[3][0:1, 0:1, 0:1])
            nc.gpsimd.dma_start(wv_sb[:],
                                wv_d[:].rearrange("w j i c a -> c w j i a"))
            nc.gpsimd.dma_start(ident_sb[:], ident_d[:])
            nc.gpsimd.dma_start(wo_sb[:], wo_d[:].rearrange("c r f -> r c f"))

        # ---- persistent state ----
        k_dr = persist.tile([128, 2, T], F8, tag="kdr")   # [(h j), i, t]
        v_sb = persist.tile([128, HPC, TB, 65], BF16, tag="v")
        nc.vector.memset(v_sb[:, :, :, 64:65], 1.0)
        y_sbT = persist.tile([128, 2, T], BF16, tag="yT")
        y_norm = persist.tile([128, TC, 4, 256], BF16, tag="ynorm")
        # per-chunk bf16 q/k rings in raw drain layout (skip the DRAM
        # shuffle for each chunk's first task; any consistent a-permutation
        # works for S). bufs=2: chunk n+1 may start before (n,0) finishes.
        bfp = ctx.enter_context(tc.tile_pool(name="bfp", bufs=2))
        qbf_tiles = [None] * TC
        kbf_tiles = [None] * TC

        # ---- x chunks: [128, 16, 512] fp8 (hi tiles 0-7, lo 8-15) ----
        xs_tiles = [None] * TC
        xs_tiles[0] = xpool.tile([128, 16, 512], F8, tag="xs", name="xs0")
        nc.sync.dma_start(
            xs_tiles[0][:, 0:4, :],
            xhl[0, 0:4, :, 0:512].rearrange("c r t -> r c t"))
        nc.sync.dma_start(
            xs_tiles[0][:, 4:8, :],
            xhl[0, 4:8, :, 0:512].rearrange("c r t -> r c t"))
        nc.sync.dma_start(
            wq_sb[:, 0, 1], wq_d[0, 1].rearrange("j i c a -> c j i a"))
        nc.scalar.dma_start(
            wk_sb[:, 0, 1], wk_d[0, 1].rearrange("j i c a -> c j i a"))
        nc.sync.dma_start(
            xs_tiles[0][:, 8:16, :],
            xhl[1, :, :, 0:512].rearrange("c r t -> r c t"))
        nc.sync.dma_start(
            wq_sb[:, 1], wq_d[1].rearrange("w j i c a -> c w j i a"))
        nc.sync.dma_start(
            wk_sb[:, 1], wk_d[1].rearrange("w j i c a -> c w j i a"))

        def load_xs(tcn):
            xs_tiles[tcn] = xpool.tile([128, 16, 512], F8, tag="xs",
                                       name=f"xs{tcn}")
            nc.sync.dma_start(
                xs_tiles[tcn][:],
                xhl[:, :, :, ts(tcn, 512)].rearrange("h c r t -> r (h c) t"),
            )

        q_tiles = [None] * TC

        def qkv_units(tcn, cp_eng=None):
            """PE filler units producing Q/K (fp8 DR layout) + V for chunk tcn.

            Returns (qk_units, v_units). qk units include the drain + the
            DRAM-shuffle DMAs; the final unit issues the q_dr/k_dr reads.
            Chunk 0 drains q/k to bf16 raw-layout tiles instead (no q
            shuffle; k still shuffles into k_dr for later chunks).
            """
            xs = xs_tiles[tcn]
            if cp_eng is None:
                cp_eng = nc.vector
            if tcn > 0:
                q_f8 = fpool.tile([128, 2, 512], F8, tag="qf8",
                                  name=f"qf8_{tcn}")
                q_scr = dscr.tile([2, 2, 32, 2, 512], F8, tag="qscr",
                                  name=f"qscr{tcn}")
                q_tiles[tcn] = qpool.tile([128, 2, 512], F8, tag="q",
                                          name=f"q{tcn}")
            else:
                q_f8 = q_scr = None
            k_f8 = fpool.tile([128, 2, 512], F8, tag="kf8", name=f"kf8_{tcn}")
            k_scr = dscr.tile([2, 2, 32, 2, 512], F8, tag="kscr",
                              name=f"kscr{tcn}")
            qbf_tiles[tcn] = bfp.tile([128, 2, 512], BF16, tag="qbf",
                                      name=f"qbf{tcn}")
            kbf_tiles[tcn] = bfp.tile([128, 2, 512], BF16, tag="kbf",
                                      name=f"kbf{tcn}")

            def qk_unit(p, w_sb, f8t, scr, is_q):
                """Returns two ~0.64us sub-units: [matmuls 1-12], [matmuls
                13-24 + drains + shuffle DMAs]."""
                state = {}
                mms = [(xh, wh, j, th) for (xh, wh) in TERMS
                       for j in range(4) for th in range(2)]

                def emit(sel, first):
                    for n, (xh, wh, j, th) in enumerate(sel):
                        nc.tensor.matmul(
                            state["t"][:, ts(th, 256)],
                            lhsT=w_sb[:, p, wh, j],
                            rhs=xs[:, 8 * xh + 2 * j: 8 * xh + 2 * j + 2,
                                   ts(th, 256)],
                            start=first and n == 0,
                            stop=(xh, wh) == TERMS[-1] and j == 3 and th == 1,
                            perf_mode=DR,
                            skip_group_check=True,
                        )

                def u_a():
                    state["t"] = po.tile([128, 512], F32, tag="po",
                                         name="po_qk")
                    emit(mms[:12], True)

                def u_b():
                    emit(mms[12:], False)
                    t = state["t"]
                    if tcn == 0 and is_q:
                        # chunk-0 q: bf16 raw-layout only (no shuffle);
                        # scalar drain - it gates the first diag exps, and
                        # Act is idle-waiting on them anyway
                        nc.scalar.copy(qbf_tiles[tcn][:, p, :], t[:])
                        return
                    if tcn == 0 or p == 0:
                        # bf16 raw-layout drain (gates the diag-first exps):
                        # on Act while it still has idle pockets (chunks 0-1),
                        # on DVE once the exp stream saturates
                        beng = nc.scalar if tcn < 2 else nc.vector
                        (beng.copy if beng is nc.scalar
                         else beng.tensor_copy)(
                            qbf_tiles[tcn][:, p, :] if is_q
                            else kbf_tiles[tcn][:, p, :], t[:])
                    nc.vector.tensor_copy(f8t[:, p, :], t[:])
                    # contiguous scratch write for this p
                    nc.sync.dma_start(
                        scr[p].rearrange("hl j i t -> (hl j i) t"),
                        f8t[:, p, :])
                    if p == 1:
                        # both halves written: read back in DR layout
                        if is_q:
                            nc.sync.dma_start(
                                q_tiles[tcn][:],
                                scr[:].rearrange("p hl j i t -> (p hl j) i t"))
                        else:
                            nc.sync.dma_start(
                                k_dr[:, :, ts(tcn, 512)],
                                scr[:].rearrange("p hl j i t -> (p hl j) i t"))
                return [u_a, u_b]

            def v_unit(tbl, eng):
                def u():
                    t = po.tile([128, 256], F32, tag="po", name="po_v")
                    first = True
                    for (xh, wh) in TERMS:
                        for j in range(4):
                            nc.tensor.matmul(
                                t[:],
                                lhsT=xs[:, 8 * xh + 2 * j: 8 * xh + 2 * j + 2,
                                        ts(tbl, 128)],
                                rhs=wv_sb[:, wh, j],
                                start=first,
                                stop=(xh, wh) == TERMS[-1] and j == 3,
                                perf_mode=DR,
                                skip_group_check=True,
                            )
                            first = False
                    cp = eng.copy if eng is nc.scalar else eng.tensor_copy
                    cp(
                        v_sb[:, :, 4 * tcn + tbl, 0:64],
                        t[:].rearrange("r (h a) -> r h a", h=HPC),
                    )
                return u

            qk_a = (qk_unit(0, wq_sb, q_f8, q_scr, True)
                    + qk_unit(0, wk_sb, k_f8, k_scr, False))
            qk_b = (qk_unit(1, wq_sb, q_f8, q_scr, True)
                    + qk_unit(1, wk_sb, k_f8, k_scr, False))
            veng = nc.vector if tcn >= 2 else cp_eng
            vs = [v_unit(tbl, veng) for tbl in range(4)]
            return qk_a, qk_b, vs

        def tr_unit(tcn, tbl, cjs, eng=None):
            """PE-transpose y_norm(tcn, tbl) column block(s) cj into y_sbT."""
            def u():
                tb = 4 * tcn + tbl
                t = po.tile([128, len(cjs), 128], BF16, tag="po", name="po_tr")
                for i, cj in enumerate(cjs):
                    nc.tensor.transpose(
                        t[:, i, :],
                        y_norm[:, tcn, tbl, ts(cj, 128)],
                        ident_sb[:],
                    )
                e = eng or nc.vector
                cp = e.copy if e is nc.scalar else e.tensor_copy
                if len(cjs) == 2:
                    cp(y_sbT[:, :, ts(tb, 128)], t[:])
                else:
                    cp(y_sbT[:, cjs[0], ts(tb, 128)], t[:, 0, :])
            return u

        def o_unit(tcn, tbl, eng=None):
            """Two sub-units (one per 512-wide half of the projection)."""
            tb = 4 * tcn + tbl
            e = eng or nc.vector
            cp = e.copy if e is nc.scalar else e.tensor_copy
            state = {}

            def half(cc):
                t = po.tile([128, 512], F32, tag="po", name="po_o")
                for cj in range(2):
                    nc.tensor.matmul(
                        t[:], lhsT=y_sbT[:, cj, ts(tb, 128)],
                        rhs=wo_sb[:, cj, ts(cc, 512)],
                        start=(cj == 0), stop=(cj == 1),
                    )
                cp(state["ot"][:, ts(cc, 512)], t[:])

            def u_a():
                state["ot"] = otp.tile([128, C], BF16, tag="ot", name="ot")
                half(0)

            def u_b():
                half(1)
                nc.sync.dma_start(out[tb][:], state["ot"][:])
            return [u_a, u_b]

        def out_units(tcn):
            units = [tr_unit(tcn, tbl, (0, 1)) for tbl in range(4)]
            for tbl in range(4):
                units += o_unit(tcn, tbl)
            return units

        # ---------------- global head-task pipeline ----------------
        # Fillers are (group, fn) pairs popped FIFO between score pairs.
        # Insertion order = dependency order; drain_fillers() force-pops a
        # group (and everything queued ahead of it) before dependent work.
        fillers = []
        av_closures = {0: [], 1: [], 2: [], 3: []}

        def add_fillers(group, units):
            fillers.extend((group, u) for u in units)

        def pop_filler():
            if fillers:
                fillers.pop(0)[1]()

        def drain_fillers(group):
            while any(g == group for g, _ in fillers):
                fillers.pop(0)[1]()

        def attn_task(tcn, h, pops=(), bf16=False, diag_first=False,
                      ring_pb=False, drain_at=None, inline_av=False):
            q_ap = q_tiles[tcn]
            npairs = 2 * tcn + 2
            pb_tiles = {}
            p_, hl_ = h // 2, h % 2
            yt = (po.tile([128, 4, 65], F32, tag="po", name="ypsi")
                  if inline_av else None)

            def sc_mm(sps, col, sb, t0, w, first, last, use_bf):
                """One score matmul: s-block sb, t range [t0, t0+w)."""
                if use_bf:
                    # bf tiles hold only this chunk: local s-block index
                    nc.tensor.matmul(
                        sps[:, col:col + w],
                        lhsT=kbf_tiles[tcn][64 * hl_:64 * hl_ + 64, p_,
                                            ts(sb - 4 * tcn, 128)],
                        rhs=qbf_tiles[tcn][64 * hl_:64 * hl_ + 64, p_,
                                           t0:t0 + w],
                        start=first, stop=last,
                        skip_group_check=True,
                    )
                    return
                nc.tensor.matmul(
                    sps[:, col:col + w],
                    lhsT=k_dr[32 * h:32 * h + 32, :, ts(sb, 128)],
                    rhs=q_ap[32 * h:32 * h + 32, :, t0:t0 + w],
                    start=first, stop=last,
                    perf_mode=DR,
                    skip_group_check=True,
                    tile_position=(32 * h, 0),
                )

            def av_emit(pi, yt, arm, finish):
                pb = pb_tiles[pi]
                if pi < 2 * tcn:
                    sbs = [(2 * pi, lambda tbl: 128 * tbl, 0),
                           (2 * pi + 1, lambda tbl: 512 + 128 * tbl, 0)]
                elif pi == 2 * tcn:
                    sbs = [(4 * tcn, lambda tbl: 128 * tbl, 0),
                           (4 * tcn + 1, lambda tbl: 384 + 128 * tbl, 1)]
                else:
                    sbs = [(4 * tcn + 2, lambda tbl: 128 * tbl - 256, 2),
                           (4 * tcn + 3, lambda tbl: 256, 3)]
                mms = [(sb, colf(tbl), tbl)
                       for sb, colf, tbl0 in sbs for tbl in range(tbl0, 4)]
                for i, (sb, c0, tbl) in enumerate(mms):
                    nc.tensor.matmul(
                        yt[:, tbl, :],
                        lhsT=pb[:, c0:c0 + 128],
                        rhs=v_sb[:, h, sb, :],
                        start=(arm and i == 0),
                        stop=(finish and i == len(mms) - 1),
                        skip_group_check=True,
                    )

            def emit_pair(pi, use_bf):
                if pi == 2 * tcn + 1 and tcn < 3:
                    sps = po.tile([128, 512], F32, tag="po", name="spsb")
                else:
                    sps = ps_s.tile([128, 1024], F32, tag="s", name="sps")
                if ring_pb:
                    pb = pbp.tile([128, 1024], BF16, name="pb", bufs=26,
                                  tag="pb")
                else:
                    pb = pbp.tile([128, 1024], BF16, name="pbd", bufs=1,
                                  tag=f"pbd{tcn}{h}{pi}")
                pb_tiles[pi] = pb
                if pi < 2 * tcn:
                    # two full 512-wide s-blocks; banks at cols 0/512
                    for jj in range(2):
                        sb = 2 * pi + jj
                        sc_mm(sps, 512 * jj, sb, 0, 256, True, False, use_bf)
                        sc_mm(sps, 512 * jj + 256, sb, 256, 256, False, True,
                              use_bf)
                    nc.scalar.activation(pb[:], sps[:], AF.Exp, scale=SCALE)
                elif pi == 2 * tcn:
                    # diagonal pair A: d0 (512 wide) + d1 (384 wide);
                    # triangle masks multiply on the (idle) Pool engine
                    sc_mm(sps, 0, 4 * tcn, 0, 256, True, False, use_bf)
                    sc_mm(sps, 256, 4 * tcn, 256, 256, False, True, use_bf)
                    sc_mm(sps, 512, 4 * tcn + 1, 128, 256, True, False, use_bf)
                    sc_mm(sps, 768, 4 * tcn + 1, 384, 128, False, True,
                          use_bf)
                    nc.scalar.activation(pb[:, 0:896], sps[:, 0:896],
                                         AF.Exp, scale=SCALE)
                    nc.gpsimd.tensor_mul(pb[:, 0:128], pb[:, 0:128],
                                         mask_sb[:])
                    nc.gpsimd.tensor_mul(pb[:, 512:640], pb[:, 512:640],
                                         mask_sb[:])
                else:
                    # diagonal pair B: d2 (256 wide) + d3 (128 wide)
                    sc_mm(sps, 0, 4 * tcn + 2, 256, 256, True, False, use_bf)
                    sc_mm(sps, 256, 4 * tcn + 3, 384, 128, False, True,
                          use_bf)
                    nc.scalar.activation(pb[:, 0:384], sps[:, 0:384],
                                         AF.Exp, scale=SCALE)
                    nc.gpsimd.tensor_mul(pb[:, 0:128], pb[:, 0:128],
                                         mask_sb[:])
                    nc.gpsimd.tensor_mul(pb[:, 256:384], pb[:, 256:384],
                                         mask_sb[:])

            if diag_first:
                order = [2 * tcn, 2 * tcn + 1] + list(range(2 * tcn))
            else:
                order = list(range(npairs))
            qka_drained = qkb_drained = False
            for ei, pi in enumerate(order):
                use_bf = bf16 or (diag_first and pi >= 2 * tcn)
                if use_bf and not qka_drained:
                    drain_fillers(f"qka{tcn}")
                    qka_drained = True
                if not use_bf and not qkb_drained:
                    drain_fillers(f"qkb{tcn}")
                    qkb_drained = True
                emit_pair(pi, use_bf)
                if inline_av and ei >= 2:
                    av_emit(order[ei - 2], yt, arm=(ei == 2), finish=False)
                if drain_at and ei in drain_at:
                    drain_fillers(drain_at[ei])
                for _ in range(pops.count(ei)):
                    pop_filler()

            def norm_emit(yt):
                rc = rcp.tile([128, 4], F32, tag="rc", name="rc")
                nc.vector.reciprocal(rc[:], yt[:, :, 64])
                nc.vector.tensor_mul(
                    y_norm[:, tcn, :, 64 * h:64 * h + 64],
                    yt[:, :, 0:64],
                    rc[:].unsqueeze(2).broadcast_to([128, 4, 64]),
                )

            if inline_av:
                av_emit(order[npairs - 2], yt, arm=False, finish=False)
                av_emit(order[npairs - 1], yt, arm=False, finish=True)
                norm_emit(yt)
            else:
                def av_closure():
                    for c in range(tcn + 1):
                        drain_fillers(f"v{c}")
                    yt2 = po.tile([128, 4, 65], F32, tag="po", name="ypsd")
                    for pi in range(npairs):
                        av_emit(pi, yt2, arm=(pi == 0),
                                finish=(pi == npairs - 1))
                    norm_emit(yt2)
                av_closures[tcn].append(av_closure)

        # ---- schedule ----
        qk0a, qk0b, v0 = qkv_units(0)
        for u in qk0a:
            u()
        attn_task(0, 0, bf16=True)
        qk0b[0]()
        qk0b[1]()
        attn_task(0, 1, bf16=True)
        qk0b[2]()
        qk0b[3]()
        load_xs(1)
        qk1a, qk1b, v1 = qkv_units(1)
        add_fillers("qka1", qk1a)
        add_fillers("qkb1", qk1b)
        load_xs(2)
        attn_task(0, 2, pops=(0, 0, 0, 1, 1, 1), bf16=True)
        qk2a, qk2b, v2 = qkv_units(2)
        add_fillers("qka2", qk2a)
        add_fillers("qkb2", qk2b)
        attn_task(0, 3, pops=(0, 0, 0, 1, 1, 1), bf16=True)
        load_xs(3)
        load_late_consts()
        qk3a, qk3b, v3 = qkv_units(3)
        attn_task(1, 0, pops=(0, 1, 2, 3), diag_first=True)
        add_fillers("qka3", qk3a)
        add_fillers("qkb3", qk3b)
        attn_task(1, 1, pops=(0, 1, 2, 3), diag_first=True)
        attn_task(1, 2, pops=(0, 1, 2, 3))
        attn_task(1, 3, pops=(0, 1, 2, 3))
        add_fillers("v0", v0)
        add_fillers("v1", v1)
        add_fillers("v2", v2)
        add_fillers("v3", v3)
        add_fillers("av0", av_closures[0])
        add_fillers("av1", av_closures[1])
        attn_task(2, 0, pops=(0, 0, 1, 1, 2, 2, 3, 4), diag_first=True,
                  ring_pb=True)
        add_fillers("cl20", av_closures[2][0:1])
        attn_task(2, 1, pops=(0, 0, 1, 1, 2, 3, 4, 5), ring_pb=True,
                  diag_first=True)
        add_fillers("cl21", av_closures[2][1:2])
        add_fillers("out0", out_units(0))
        attn_task(2, 2, pops=(0, 0, 1, 1, 2, 3, 4, 5), ring_pb=True)
        add_fillers("cl22", av_closures[2][2:3])
        attn_task(3, 0, pops=(0, 1, 2, 3, 4, 5, 6, 7), diag_first=True,
                  ring_pb=True)
        add_fillers("cl30", av_closures[3][0:1])
        add_fillers("out1", out_units(1))
        attn_task(2, 3, pops=(0, 0, 1, 1, 2, 3, 4, 5), ring_pb=True)
        add_fillers("cl23", av_closures[2][3:4])
        add_fillers("out2", out_units(2))
        attn_task(3, 1, pops=(0, 1, 2, 3, 4, 5, 6, 7), ring_pb=True,
                  diag_first=True)
        add_fillers("cl31", av_closures[3][1:2])
        add_fillers("tr3", [tr_unit(3, tbl, (0,)) for tbl in range(4)])
        attn_task(3, 2, pops=(0, 1, 2, 3, 4, 5, 6, 7), ring_pb=True)
        add_fillers("cl32", av_closures[3][2:3])
        attn_task(3, 3, pops=(0, 1, 2, 3, 4, 5, 6, 7), ring_pb=True)
        add_fillers("cl33", av_closures[3][3:4])
        while fillers:
            pop_filler()
        # chunk-3 tail: all second-half transposes first (drains alternate
        # DVE/Act), then the output projections, so the blocks pipeline
        t1 = [tr_unit(3, tbl, (1,), eng=(nc.scalar if tbl % 2 else None))
              for tbl in range(4)]
        for u in t1:
            u()
        for tbl in range(4):
            for u in o_unit(3, tbl, eng=(nc.scalar if tbl % 2 else None)):
                u()

    nc.compile()
    return nc


_NC = None


def _get_nc():
    global _NC
    if _NC is None:
        _NC = build_nc()
    return _NC


def _mask_arr():
    p = np.arange(128)[:, None]
    f = np.arange(128)[None, :]
    return np.ascontiguousarray((p <= f).astype(NBF))


def _hilo(a, s):
    hi = (s * a).astype(NF8)
    lo = ((s * a) - hi.astype(np.float32)).astype(NF8)
    return hi, lo


def _pack_qk(w, hs):
    """w[H,C,Ca] -> [2(whalf), 2(pair... ) ...] per-core DR pack.

    Layout [w, j, i, c(128), a(128)] per p, with a-columns ordered
    (hl, jj, ii): column m = 64*hl + 2*jj + ii -> head hs[2p+hl] channel
    a = 32*ii + jj.
    """
    out = np.zeros((2, 2, 4, 2, 128, 128), NF8)  # [p, w, pair, tile, c, a]
    m = np.arange(128)
    hl, r = m // 64, m % 64
    jj, ii = r // 2, r % 2
    a_of_m = 32 * ii + jj
    for p in range(2):
        cols = np.stack([w[hs[2 * p + hl_], :, a_] for hl_, a_ in
                         zip(hl, a_of_m)], axis=1)  # [C, 128]
        hi, lo = _hilo(cols, SW)
        for wh, arr in ((0, hi), (1, lo)):
            out[p, wh] = arr.reshape(4, 2, 128, 128)
    return np.ascontiguousarray(out)


def make_in_maps(x, w_q, w_k, w_v, w_o):
    x = np.asarray(x, dtype=np.float32)
    w_q = np.asarray(w_q, dtype=np.float32)
    w_k = np.asarray(w_k, dtype=np.float32)
    w_v = np.asarray(w_v, dtype=np.float32)
    w_o = np.asarray(w_o, dtype=np.float32)
    in_maps = []
    xhl_b = []
    for b in range(B):
        xT = np.ascontiguousarray(x[b].T)                    # [C, T]
        xhi, xlo = _hilo(xT, SX)
        xhl_b.append(np.ascontiguousarray(
            np.stack([xhi, xlo]).reshape(2, CK, 128, T)))
    for c in range(NCORES):
        b, g = c // 4, c % 4
        hs = [4 * g + i for i in range(HPC)]
        wv_cols = np.concatenate([w_v[h] for h in hs], axis=1)  # [C, 256]
        wv_hi, wv_lo = _hilo(wv_cols, SW)
        wv_a = np.stack([wv_hi.reshape(4, 2, 128, 256),
                         wv_lo.reshape(4, 2, 128, 256)])
        wo_a = (w_o[256 * g:256 * (g + 1)] / (SX * SW)).reshape(
            2, 128, C).astype(NBF)
        in_maps.append(dict(
            mask=_mask_arr(),
            ident=np.ascontiguousarray(np.eye(128).astype(NBF)),
            xhl=xhl_b[b],
            wq=_pack_qk(w_q, hs),
            wk=_pack_qk(w_k, hs),
            wv=np.ascontiguousarray(wv_a),
            wo=np.ascontiguousarray(wo_a),
        ))
    return in_maps


def gather_out(results):
    acc = [np.zeros((T, C), np.float64) for _ in range(B)]
    for c in range(NCORES):
        acc[c // 4] += results[c]["out"].reshape(T, C).astype(np.float64)
    return np.stack([a.astype(np.float32) for a in acc])


def run(x, w_q, w_k, w_v, w_o, trace=False, **spmd_kwargs):
    nc = _get_nc()
    in_maps = make_in_maps(x, w_q, w_k, w_v, w_o)
    res = run_bass_kernel_spmd(nc, in_maps, list(range(NCORES)), trace=trace,
                               **spmd_kwargs)
    return gather_out(res.results), res


def kernel(x, w_q, w_k, w_v, w_o):
    out, _ = run(x, w_q, w_k, w_v, w_o)
    return out


# revision 8
# speedup vs baseline: 1.0233x; 1.0228x over previous
"""Causal MHA (B=2,T=2048,C=1024,H=16,Ca=64) on 8 trn2 cores - fp8 DoubleRow v2.

Sharding: core c gets batch b=c//4, heads [4g,4g+4), g=c%4; partial output
projections through 256-row w_o slices summed on host (bf16 partials).

Design (vs the f32r/bf16 chunk-pipelined baseline):
  - QKV projections via fp8e4 DoubleRow matmuls (2 k-tiles per PE pass):
    the host pre-splits x (x4 scale) and w_q/w_k/w_v (x16 scale) into fp8
    hi/lo pairs; 3 cross terms (hi.hi + hi.lo + lo.hi) recover ~f32r
    accuracy at 3/4 of the f32r PE cost. Scales avoid fp8 subnormals; the
    exp scale and w_o absorb them exactly.
  - Scores via fp8 DoubleRow with Ca=64 split 2x32: Q/K drain psum->fp8,
    then DMA through a DRAM scratch into the [32(j),2(i),t] layout (w
    columns host-ordered (hl,j,i) making the scratch writes contiguous).
    Half the bf16 score cost.
  - Each chunk's first two head tasks emit their diagonal score pairs
    first from bf16 raw-drain-layout tiles (no shuffle wait), keeping the
    Act engine's exp stream fed across chunk boundaries while the fp8
    shuffle completes.
  - Triangle masks multiply post-exp on the idle GpSimd engine.
  - AV stays bf16 natural-layout with a ones column emitting softmax row
    sums; AV+normalization run as deferred closures popped as PE fillers
    in the Act-bound second half, with a pb ring for chunks 2-3.
  - All PE work is cut into ~0.6us filler sub-units popped between score
    pairs under a group FIFO with force-drain for cross dependencies.
  - Output projection bf16 (y^T via PE transposes); bf16 output partials.
"""

import math
import sys

import numpy as np

for _p in ("/opt/trn_rl_repo",):
    if _p not in sys.path:
        sys.path.insert(0, _p)

import ml_dtypes
import concourse.bass as bass
from concourse import bacc
import concourse.mybir as mybir
from concourse.bass import ts
from concourse.tile import TileContext
from concourse.bass_utils import run_bass_kernel_spmd
from contextlib import ExitStack

F32 = mybir.dt.float32
BF16 = mybir.dt.bfloat16
F8 = mybir.dt.float8e4
DR = mybir.MatmulPerfMode.DoubleRow
AF = mybir.ActivationFunctionType
NF8 = ml_dtypes.float8_e4m3
NBF = ml_dtypes.bfloat16

B, T, C = 2, 2048, 1024
H, CA = 16, 64
NCORES = 8
HPC = 4          # heads per core
TB = T // 128    # 16 t-blocks
TC = T // 512    # 4 t-chunks
CK = C // 128    # 8 c-tiles
SX, SW = 4.0, 16.0           # host fp8 pre-scales for x and w
SCALE = (1.0 / math.sqrt(CA)) / (SX * SX * SW * SW)  # exp scale absorbs both
TERMS = ((0, 0), (0, 1), (1, 0))  # (x half, w half): hi*hi + hi*lo + lo*hi


def build_nc():
    nc = bacc.Bacc()
    xhl = nc.declare_dram_parameter("xhl", [2, CK, 128, T], F8, isOutput=False)
    wq_d = nc.declare_dram_parameter("wq", [2, 2, 4, 2, 128, 128], F8, isOutput=False)
    wk_d = nc.declare_dram_parameter("wk", [2, 2, 4, 2, 128, 128], F8, isOutput=False)
    wv_d = nc.declare_dram_parameter("wv", [2, 4, 2, 128, 256], F8, isOutput=False)
    wo_d = nc.declare_dram_parameter("wo", [2, 128, C], BF16, isOutput=False)
    mask_d = nc.declare_dram_parameter("mask", [128, 128], BF16, isOutput=False)
    ident_d = nc.declare_dram_parameter("ident", [128, 128], BF16, isOutput=False)
    out = nc.declare_dram_parameter("out", [TB, 128, C], BF16, isOutput=True)

    with TileContext(nc) as tc, ExitStack() as ctx:
        const = ctx.enter_context(tc.tile_pool(name="const", bufs=1))
        persist = ctx.enter_context(tc.tile_pool(name="persist", bufs=1))
        qpool = ctx.enter_context(tc.tile_pool(name="qpool", bufs=3))
        fpool = ctx.enter_context(tc.tile_pool(name="fpool", bufs=2))
        xpool = ctx.enter_context(tc.tile_pool(name="xpool", bufs=4))
        dscr = ctx.enter_context(tc.tile_pool(name="dscr", bufs=2, space="DRAM"))
        pbp = ctx.enter_context(tc.tile_pool(name="pbp", bufs=4))
        rcp = ctx.enter_context(tc.tile_pool(name="rcp", bufs=2))
        otp = ctx.enter_context(tc.tile_pool(name="otp", bufs=2))
        ps_s = ctx.enter_context(tc.tile_pool(name="ps_s", bufs=2, space="PSUM"))
        po = ctx.enter_context(tc.tile_pool(name="po", bufs=4, space="PSUM"))

        # ---- constants / weights: startup-latency ordered ----
        wq_sb = const.tile([128, 2, 2, 4, 2, 128], F8, tag="wq")
        wk_sb = const.tile([128, 2, 2, 4, 2, 128], F8, tag="wk")
        # first-use order: wq-p0-hi, wk-p0-hi, then x chunk 0, then the rest
        nc.sync.dma_start(wq_sb[:, 0, 0], wq_d[0, 0].rearrange("j i c a -> c j i a"))
        nc.scalar.dma_start(wk_sb[:, 0, 0], wk_d[0, 0].rearrange("j i c a -> c j i a"))
        wv_sb = const.tile([128, 2, 4, 2, 256], F8, tag="wv")
        mask_sb = const.tile([128, 128], BF16, tag="mask")
        nc.scalar.dma_start(mask_sb[:], mask_d[:])
        ident_sb = const.tile([128, 128], BF16, tag="ident")
        wo_sb = const.tile([128, 2, C], BF16, tag="wo")

        dly = persist.tile([1, 4], F8, tag="dly")

        def load_late_consts():
            # Pool-queue DMAs, held back by a 1-element copy that reads the
            # xs3 tile: Pool program order then defers the transfers past
            # the x streams (v units / o-proj consume these much later)
            nc.gpsimd.tensor_copy(dly[:, 0:1], xs_tiles[3][0:1, 0:1, 0:1])
            nc.gpsimd.dma_start(wv_sb[:],
                                wv_d[:].rearrange("w j i c a -> c w j i a"))
            nc.gpsimd.dma_start(ident_sb[:], ident_d[:])
            nc.gpsimd.dma_start(wo_sb[:], wo_d[:].rearrange("c r f -> r c f"))

        # ---- persistent state ----
        k_dr = persist.tile([128, 2, T], F8, tag="kdr")   # [(h j), i, t]
        v_sb = persist.tile([128, HPC, TB, 65], BF16, tag="v")
        nc.vector.memset(v_sb[:, :, :, 64:65], 1.0)
        y_sbT = persist.tile([128, 2, T], BF16, tag="yT")
        y_norm = persist.tile([128, TC, 4, 256], BF16, tag="ynorm")
        # per-chunk bf16 q/k rings in raw drain layout (skip the DRAM
        # shuffle for each chunk's first task; any consistent a-permutation
        # works for S). bufs=2: chunk n+1 may start before (n,0) finishes.
        bfp = ctx.enter_context(tc.tile_pool(name="bfp", bufs=2))
        qbf_tiles = [None] * TC
        kbf_tiles = [None] * TC

        # ---- x chunks: [128, 16, 512] fp8 (hi tiles 0-7, lo 8-15) ----
        xs_tiles = [None] * TC
        xs_tiles[0] = xpool.tile([128, 16, 512], F8, tag="xs", name="xs0")
        nc.sync.dma_start(
            xs_tiles[0][:, 0:4, :],
            xhl[0, 0:4, :, 0:512].rearrange("c r t -> r c t"))
        nc.sync.dma_start(
            xs_tiles[0][:, 4:8, :],
            xhl[0, 4:8, :, 0:512].rearrange("c r t -> r c t"))
        nc.sync.dma_start(
            wq_sb[:, 0, 1], wq_d[0, 1].rearrange("j i c a -> c j i a"))
        nc.scalar.dma_start(
            wk_sb[:, 0, 1], wk_d[0, 1].rearrange("j i c a -> c j i a"))
        nc.sync.dma_start(
            xs_tiles[0][:, 8:16, :],
            xhl[1, :, :, 0:512].rearrange("c r t -> r c t"))
        nc.sync.dma_start(
            wq_sb[:, 1], wq_d[1].rearrange("w j i c a -> c w j i a"))
        nc.sync.dma_start(
            wk_sb[:, 1], wk_d[1].rearrange("w j i c a -> c w j i a"))

        def load_xs(tcn):
            xs_tiles[tcn] = xpool.tile([128, 16, 512], F8, tag="xs",
                                       name=f"xs{tcn}")
            # later chunks via the idle Pool SWDGE queue: the transfers
            # stop serializing ahead of the q/k shuffle DMAs in SP program
            # order (queue reassignment only - emission order unchanged)
            eng = nc.sync if tcn == 1 else nc.gpsimd
            eng.dma_start(
                xs_tiles[tcn][:],
                xhl[:, :, :, ts(tcn, 512)].rearrange("h c r t -> r (h c) t"),
            )

        q_tiles = [None] * TC

        def qkv_units(tcn, cp_eng=None):
            """PE filler units producing Q/K (fp8 DR layout) + V for chunk tcn.

            Returns (qk_units, v_units). qk units include the drain + the
            DRAM-shuffle DMAs; the final unit issues the q_dr/k_dr reads.
            Chunk 0 drains q/k to bf16 raw-layout tiles instead (no q
            shuffle; k still shuffles into k_dr for later chunks).
            """
            xs = xs_tiles[tcn]
            if cp_eng is None:
                cp_eng = nc.vector
            if tcn > 0:
                q_f8 = fpool.tile([128, 2, 512], F8, tag="qf8",
                                  name=f"qf8_{tcn}")
                q_scr = dscr.tile([2, 2, 32, 2, 512], F8, tag="qscr",
                                  name=f"qscr{tcn}")
                q_tiles[tcn] = qpool.tile([128, 2, 512], F8, tag="q",
                                          name=f"q{tcn}")
            else:
                q_f8 = q_scr = None
            k_f8 = fpool.tile([128, 2, 512], F8, tag="kf8", name=f"kf8_{tcn}")
            k_scr = dscr.tile([2, 2, 32, 2, 512], F8, tag="kscr",
                              name=f"kscr{tcn}")
            qbf_tiles[tcn] = bfp.tile([128, 2, 512], BF16, tag="qbf",
                                      name=f"qbf{tcn}")
            kbf_tiles[tcn] = bfp.tile([128, 2, 512], BF16, tag="kbf",
                                      name=f"kbf{tcn}")

            def qk_unit(p, w_sb, f8t, scr, is_q):
                """Returns two ~0.64us sub-units: [matmuls 1-12], [matmuls
                13-24 + drains + shuffle DMAs]."""
                state = {}
                mms = [(xh, wh, j, th) for (xh, wh) in TERMS
                       for j in range(4) for th in range(2)]

                def emit(sel, first):
                    for n, (xh, wh, j, th) in enumerate(sel):
                        nc.tensor.matmul(
                            state["t"][:, ts(th, 256)],
                            lhsT=w_sb[:, p, wh, j],
                            rhs=xs[:, 8 * xh + 2 * j: 8 * xh + 2 * j + 2,
                                   ts(th, 256)],
                            start=first and n == 0,
                            stop=(xh, wh) == TERMS[-1] and j == 3 and th == 1,
                            perf_mode=DR,
                            skip_group_check=True,
                        )

                def u_a():
                    state["t"] = po.tile([128, 512], F32, tag="po",
                                         name="po_qk")
                    emit(mms[:12], True)

                def u_b():
                    emit(mms[12:], False)
                    t = state["t"]
                    if tcn == 0 and is_q:
                        # chunk-0 q: bf16 raw-layout only (no shuffle);
                        # scalar drain - it gates the first diag exps, and
                        # Act is idle-waiting on them anyway
                        nc.scalar.copy(qbf_tiles[tcn][:, p, :], t[:])
                        return
                    if tcn == 0 or p == 0:
                        # bf16 raw-layout drain (gates the diag-first exps):
                        # on Act while it still has idle pockets (chunks 0-1),
                        # on DVE once the exp stream saturates
                        beng = nc.scalar if tcn < 2 else nc.vector
                        (beng.copy if beng is nc.scalar
                         else beng.tensor_copy)(
                            qbf_tiles[tcn][:, p, :] if is_q
                            else kbf_tiles[tcn][:, p, :], t[:])
                    nc.vector.tensor_copy(f8t[:, p, :], t[:])
                    # contiguous scratch write for this p (late chunks via
                    # Pool SWDGE - off the SP queue and the global HWDGE)
                    deng = nc.sync if tcn < 2 else nc.gpsimd
                    deng.dma_start(
                        scr[p].rearrange("hl j i t -> (hl j i) t"),
                        f8t[:, p, :])
                    if p == 1:
                        # both halves written: read back in DR layout
                        if is_q:
                            deng.dma_start(
                                q_tiles[tcn][:],
                                scr[:].rearrange("p hl j i t -> (p hl j) i t"))
                        else:
                            deng.dma_start(
                                k_dr[:, :, ts(tcn, 512)],
                                scr[:].rearrange("p hl j i t -> (p hl j) i t"))
                return [u_a, u_b]

            def v_unit(tbl, eng):
                def u():
                    t = po.tile([128, 256], F32, tag="po", name="po_v")
                    first = True
                    for (xh, wh) in TERMS:
                        for j in range(4):
                            nc.tensor.matmul(
                                t[:],
                                lhsT=xs[:, 8 * xh + 2 * j: 8 * xh + 2 * j + 2,
                                        ts(tbl, 128)],
                                rhs=wv_sb[:, wh, j],
                                start=first,
                                stop=(xh, wh) == TERMS[-1] and j == 3,
                                perf_mode=DR,
                                skip_group_check=True,
                            )
                            first = False
                    cp = eng.copy if eng is nc.scalar else eng.tensor_copy
                    cp(
                        v_sb[:, :, 4 * tcn + tbl, 0:64],
                        t[:].rearrange("r (h a) -> r h a", h=HPC),
                    )
                return u

            qk_a = (qk_unit(0, wq_sb, q_f8, q_scr, True)
                    + qk_unit(0, wk_sb, k_f8, k_scr, False))
            qk_b = (qk_unit(1, wq_sb, q_f8, q_scr, True)
                    + qk_unit(1, wk_sb, k_f8, k_scr, False))
            veng = nc.vector if tcn >= 2 else cp_eng
            vs = [v_unit(tbl, veng) for tbl in range(4)]
            return qk_a, qk_b, vs

        def tr_unit(tcn, tbl, cjs, eng=None):
            """PE-transpose y_norm(tcn, tbl) column block(s) cj into y_sbT."""
            def u():
                tb = 4 * tcn + tbl
                t = po.tile([128, len(cjs), 128], BF16, tag="po", name="po_tr")
                for i, cj in enumerate(cjs):
                    nc.tensor.transpose(
                        t[:, i, :],
                        y_norm[:, tcn, tbl, ts(cj, 128)],
                        ident_sb[:],
                    )
                e = eng or nc.vector
                cp = e.copy if e is nc.scalar else e.tensor_copy
                if len(cjs) == 2:
                    cp(y_sbT[:, :, ts(tb, 128)], t[:])
                else:
                    cp(y_sbT[:, cjs[0], ts(tb, 128)], t[:, 0, :])
            return u

        def o_unit(tcn, tbl, eng=None):
            """Two sub-units (one per 512-wide half of the projection)."""
            tb = 4 * tcn + tbl
            e = eng or nc.vector
            cp = e.copy if e is nc.scalar else e.tensor_copy
            state = {}

            def half(cc):
                t = po.tile([128, 512], F32, tag="po", name="po_o")
                for cj in range(2):
                    nc.tensor.matmul(
                        t[:], lhsT=y_sbT[:, cj, ts(tb, 128)],
                        rhs=wo_sb[:, cj, ts(cc, 512)],
                        start=(cj == 0), stop=(cj == 1),
                    )
                cp(state["ot"][:, ts(cc, 512)], t[:])

            def u_a():
                state["ot"] = otp.tile([128, C], BF16, tag="ot", name="ot")
                half(0)

            def u_b():
                half(1)
                nc.sync.dma_start(out[tb][:], state["ot"][:])
            return [u_a, u_b]

        def out_units(tcn):
            units = [tr_unit(tcn, tbl, (0, 1)) for tbl in range(4)]
            for tbl in range(4):
                units += o_unit(tcn, tbl)
            return units

        # ---------------- global head-task pipeline ----------------
        # Fillers are (group, fn) pairs popped FIFO between score pairs.
        # Insertion order = dependency order; drain_fillers() force-pops a
        # group (and everything queued ahead of it) before dependent work.
        fillers = []
        av_closures = {0: [], 1: [], 2: [], 3: []}

        def add_fillers(group, units):
            fillers.extend((group, u) for u in units)

        def pop_filler():
            if fillers:
                fillers.pop(0)[1]()

        def drain_fillers(group):
            while any(g == group for g, _ in fillers):
                fillers.pop(0)[1]()

        def attn_task(tcn, h, pops=(), bf16=False, diag_first=False,
                      ring_pb=False, drain_at=None, inline_av=False):
            q_ap = q_tiles[tcn]
            npairs = 2 * tcn + 2
            pb_tiles = {}
            p_, hl_ = h // 2, h % 2
            yt = (po.tile([128, 4, 65], F32, tag="po", name="ypsi")
                  if inline_av else None)

            def sc_mm(sps, col, sb, t0, w, first, last, use_bf):
                """One score matmul: s-block sb, t range [t0, t0+w)."""
                if use_bf:
                    # bf tiles hold only this chunk: local s-block index
                    nc.tensor.matmul(
                        sps[:, col:col + w],
                        lhsT=kbf_tiles[tcn][64 * hl_:64 * hl_ + 64, p_,
                                            ts(sb - 4 * tcn, 128)],
                        rhs=qbf_tiles[tcn][64 * hl_:64 * hl_ + 64, p_,
                                           t0:t0 + w],
                        start=first, stop=last,
                        skip_group_check=True,
                    )
                    return
                nc.tensor.matmul(
                    sps[:, col:col + w],
                    lhsT=k_dr[32 * h:32 * h + 32, :, ts(sb, 128)],
                    rhs=q_ap[32 * h:32 * h + 32, :, t0:t0 + w],
                    start=first, stop=last,
                    perf_mode=DR,
                    skip_group_check=True,
                    tile_position=(32 * h, 0),
                )

            def av_emit(pi, yt, arm, finish):
                pb = pb_tiles[pi]
                if pi < 2 * tcn:
                    sbs = [(2 * pi, lambda tbl: 128 * tbl, 0),
                           (2 * pi + 1, lambda tbl: 512 + 128 * tbl, 0)]
                elif pi == 2 * tcn:
                    sbs = [(4 * tcn, lambda tbl: 128 * tbl, 0),
                           (4 * tcn + 1, lambda tbl: 384 + 128 * tbl, 1)]
                else:
                    sbs = [(4 * tcn + 2, lambda tbl: 128 * tbl - 256, 2),
                           (4 * tcn + 3, lambda tbl: 256, 3)]
                mms = [(sb, colf(tbl), tbl)
                       for sb, colf, tbl0 in sbs for tbl in range(tbl0, 4)]
                for i, (sb, c0, tbl) in enumerate(mms):
                    nc.tensor.matmul(
                        yt[:, tbl, :],
                        lhsT=pb[:, c0:c0 + 128],
                        rhs=v_sb[:, h, sb, :],
                        start=(arm and i == 0),
                        stop=(finish and i == len(mms) - 1),
                        skip_group_check=True,
                    )

            def emit_pair(pi, use_bf):
                if pi == 2 * tcn + 1 and tcn < 3:
                    sps = po.tile([128, 512], F32, tag="po", name="spsb")
                else:
                    sps = ps_s.tile([128, 1024], F32, tag="s", name="sps")
                if ring_pb:
                    pb = pbp.tile([128, 1024], BF16, name="pb", bufs=26,
                                  tag="pb")
                else:
                    pb = pbp.tile([128, 1024], BF16, name="pbd", bufs=1,
                                  tag=f"pbd{tcn}{h}{pi}")
                pb_tiles[pi] = pb
                if pi < 2 * tcn:
                    # two full 512-wide s-blocks; banks at cols 0/512
                    for jj in range(2):
                        sb = 2 * pi + jj
                        sc_mm(sps, 512 * jj, sb, 0, 256, True, False, use_bf)
                        sc_mm(sps, 512 * jj + 256, sb, 256, 256, False, True,
                              use_bf)
                    nc.scalar.activation(pb[:], sps[:], AF.Exp, scale=SCALE)
                elif pi == 2 * tcn:
                    # diagonal pair A: d0 (512 wide) + d1 (384 wide);
                    # triangle masks multiply on the (idle) Pool engine
                    sc_mm(sps, 0, 4 * tcn, 0, 256, True, False, use_bf)
                    sc_mm(sps, 256, 4 * tcn, 256, 256, False, True, use_bf)
                    sc_mm(sps, 512, 4 * tcn + 1, 128, 256, True, False, use_bf)
                    sc_mm(sps, 768, 4 * tcn + 1, 384, 128, False, True,
                          use_bf)
                    nc.scalar.activation(pb[:, 0:896], sps[:, 0:896],
                                         AF.Exp, scale=SCALE)
                    nc.gpsimd.tensor_mul(pb[:, 0:128], pb[:, 0:128],
                                         mask_sb[:])
                    nc.gpsimd.tensor_mul(pb[:, 512:640], pb[:, 512:640],
                                         mask_sb[:])
                else:
                    # diagonal pair B: d2 (256 wide) + d3 (128 wide)
                    sc_mm(sps, 0, 4 * tcn + 2, 256, 256, True, False, use_bf)
                    sc_mm(sps, 256, 4 * tcn + 3, 384, 128, False, True,
                          use_bf)
                    nc.scalar.activation(pb[:, 0:384], sps[:, 0:384],
                                         AF.Exp, scale=SCALE)
                    nc.gpsimd.tensor_mul(pb[:, 0:128], pb[:, 0:128],
                                         mask_sb[:])
                    nc.gpsimd.tensor_mul(pb[:, 256:384], pb[:, 256:384],
                                         mask_sb[:])

            if diag_first:
                order = [2 * tcn, 2 * tcn + 1] + list(range(2 * tcn))
            else:
                order = list(range(npairs))
            qka_drained = qkb_drained = False
            for ei, pi in enumerate(order):
                use_bf = bf16 or (diag_first and pi >= 2 * tcn)
                if use_bf and not qka_drained:
                    drain_fillers(f"qka{tcn}")
                    qka_drained = True
                if not use_bf and not qkb_drained:
                    drain_fillers(f"qkb{tcn}")
                    qkb_drained = True
                emit_pair(pi, use_bf)
                if inline_av and ei >= 2:
                    av_emit(order[ei - 2], yt, arm=(ei == 2), finish=False)
                if drain_at and ei in drain_at:
                    drain_fillers(drain_at[ei])
                for _ in range(pops.count(ei)):
                    pop_filler()

            def norm_emit(yt):
                rc = rcp.tile([128, 4], F32, tag="rc", name="rc")
                nc.vector.reciprocal(rc[:], yt[:, :, 64])
                nc.vector.tensor_mul(
                    y_norm[:, tcn, :, 64 * h:64 * h + 64],
                    yt[:, :, 0:64],
                    rc[:].unsqueeze(2).broadcast_to([128, 4, 64]),
                )

            if inline_av:
                av_emit(order[npairs - 2], yt, arm=False, finish=False)
                av_emit(order[npairs - 1], yt, arm=False, finish=True)
                norm_emit(yt)
            else:
                def av_closure():
                    for c in range(tcn + 1):
                        drain_fillers(f"v{c}")
                    yt2 = po.tile([128, 4, 65], F32, tag="po", name="ypsd")
                    for pi in range(npairs):
                        av_emit(pi, yt2, arm=(pi == 0),
                                finish=(pi == npairs - 1))
                    norm_emit(yt2)
                av_closures[tcn].append(av_closure)

        # ---- schedule ----
        qk0a, qk0b, v0 = qkv_units(0)
        for u in qk0a:
            u()
        attn_task(0, 0, bf16=True)
        qk0b[0]()
        qk0b[1]()
        attn_task(0, 1, bf16=True)
        qk0b[2]()
        qk0b[3]()
        load_xs(1)
        qk1a, qk1b, v1 = qkv_units(1)
        add_fillers("qka1", qk1a)
        add_fillers("qkb1", qk1b)
        load_xs(2)
        attn_task(0, 2, pops=(0, 0, 0, 1, 1, 1), bf16=True)
        qk2a, qk2b, v2 = qkv_units(2)
        add_fillers("qka2", qk2a)
        add_fillers("qkb2", qk2b)
        attn_task(0, 3, pops=(0, 0, 0, 1, 1, 1), bf16=True)
        load_xs(3)
        load_late_consts()
        qk3a, qk3b, v3 = qkv_units(3)
        attn_task(1, 0, pops=(0, 1, 2, 3), diag_first=True)
        add_fillers("qka3", qk3a)
        add_fillers("qkb3", qk3b)
        attn_task(1, 1, pops=(0, 1, 2, 3), diag_first=True)
        attn_task(1, 2, pops=(0, 1, 2, 3))
        attn_task(1, 3, pops=(0, 1, 2, 3))
        add_fillers("v0", v0)
        add_fillers("v1", v1)
        add_fillers("v2", v2)
        add_fillers("v3", v3)
        add_fillers("av0", av_closures[0])
        add_fillers("av1", av_closures[1])
        attn_task(2, 0, pops=(0, 0, 1, 1, 2, 2, 3, 4), diag_first=True,
                  ring_pb=True)
        add_fillers("cl20", av_closures[2][0:1])
        attn_task(2, 1, pops=(0, 0, 1, 1, 2, 3, 4, 5), ring_pb=True,
                  diag_first=True)
        add_fillers("cl21", av_closures[2][1:2])
        add_fillers("out0", out_units(0))
        attn_task(2, 2, pops=(0, 0, 1, 1, 2, 3, 4, 5), ring_pb=True)
        add_fillers("cl22", av_closures[2][2:3])
        attn_task(3, 0, pops=(0, 1, 2, 3, 4, 5, 6, 7), diag_first=True,
                  ring_pb=True)
        add_fillers("cl30", av_closures[3][0:1])
        add_fillers("out1", out_units(1))
        attn_task(2, 3, pops=(0, 0, 1, 1, 2, 3, 4, 5), ring_pb=True)
        add_fillers("cl23", av_closures[2][3:4])
        add_fillers("out2", out_units(2))
        attn_task(3, 1, pops=(0, 1, 2, 3, 4, 5, 6, 7), ring_pb=True,
                  diag_first=True)
        add_fillers("cl31", av_closures[3][1:2])
        add_fillers("tr3", [tr_unit(3, tbl, (0,)) for tbl in range(4)])
        attn_task(3, 2, pops=(0, 1, 2, 3, 4, 5, 6, 7), ring_pb=True)
        add_fillers("cl32", av_closures[3][2:3])
        attn_task(3, 3, pops=(0, 1, 2, 3, 4, 5, 6, 7), ring_pb=True)
        add_fillers("cl33", av_closures[3][3:4])
        while fillers:
            pop_filler()
        # chunk-3 tail: all second-half transposes first (drains alternate
        # DVE/Act), then the output projections, so the blocks pipeline
        t1 = [tr_unit(3, tbl, (1,), eng=(nc.scalar if tbl % 2 else None))
              for tbl in range(4)]
        for u in t1:
            u()
        for tbl in range(4):
            for u in o_unit(3, tbl, eng=(nc.scalar if tbl % 2 else None)):
                u()

    nc.compile()
    return nc


_NC = None


def _get_nc():
    global _NC
    if _NC is None:
        _NC = build_nc()
    return _NC


def _mask_arr():
    p = np.arange(128)[:, None]
    f = np.arange(128)[None, :]
    return np.ascontiguousarray((p <= f).astype(NBF))


def _hilo(a, s):
    hi = (s * a).astype(NF8)
    lo = ((s * a) - hi.astype(np.float32)).astype(NF8)
    return hi, lo


def _pack_qk(w, hs):
    """w[H,C,Ca] -> [2(whalf), 2(pair... ) ...] per-core DR pack.

    Layout [w, j, i, c(128), a(128)] per p, with a-columns ordered
    (hl, jj, ii): column m = 64*hl + 2*jj + ii -> head hs[2p+hl] channel
    a = 32*ii + jj.
    """
    out = np.zeros((2, 2, 4, 2, 128, 128), NF8)  # [p, w, pair, tile, c, a]
    m = np.arange(128)
    hl, r = m // 64, m % 64
    jj, ii = r // 2, r % 2
    a_of_m = 32 * ii + jj
    for p in range(2):
        cols = np.stack([w[hs[2 * p + hl_], :, a_] for hl_, a_ in
                         zip(hl, a_of_m)], axis=1)  # [C, 128]
        hi, lo = _hilo(cols, SW)
        for wh, arr in ((0, hi), (1, lo)):
            out[p, wh] = arr.reshape(4, 2, 128, 128)
    return np.ascontiguousarray(out)


def make_in_maps(x, w_q, w_k, w_v, w_o):
    x = np.asarray(x, dtype=np.float32)
    w_q = np.asarray(w_q, dtype=np.float32)
    w_k = np.asarray(w_k, dtype=np.float32)
    w_v = np.asarray(w_v, dtype=np.float32)
    w_o = np.asarray(w_o, dtype=np.float32)
    in_maps = []
    xhl_b = []
    for b in range(B):
        xT = np.ascontiguousarray(x[b].T)                    # [C, T]
        xhi, xlo = _hilo(xT, SX)
        xhl_b.append(np.ascontiguousarray(
            np.stack([xhi, xlo]).reshape(2, CK, 128, T)))
    for c in range(NCORES):
        b, g = c // 4, c % 4
        hs = [4 * g + i for i in range(HPC)]
        wv_cols = np.concatenate([w_v[h] for h in hs], axis=1)  # [C, 256]
        wv_hi, wv_lo = _hilo(wv_cols, SW)
        wv_a = np.stack([wv_hi.reshape(4, 2, 128, 256),
                         wv_lo.reshape(4, 2, 128, 256)])
        wo_a = (w_o[256 * g:256 * (g + 1)] / (SX * SW)).reshape(
            2, 128, C).astype(NBF)
        in_maps.append(dict(
            mask=_mask_arr(),
            ident=np.ascontiguousarray(np.eye(128).astype(NBF)),
            xhl=xhl_b[b],
            wq=_pack_qk(w_q, hs),
            wk=_pack_qk(w_k, hs),
            wv=np.ascontiguousarray(wv_a),
            wo=np.ascontiguousarray(wo_a),
        ))
    return in_maps


def gather_out(results):
    acc = [np.zeros((T, C), np.float64) for _ in range(B)]
    for c in range(NCORES):
        acc[c // 4] += results[c]["out"].reshape(T, C).astype(np.float64)
    return np.stack([a.astype(np.float32) for a in acc])


def run(x, w_q, w_k, w_v, w_o, trace=False, **spmd_kwargs):
    nc = _get_nc()
    in_maps = make_in_maps(x, w_q, w_k, w_v, w_o)
    res = run_bass_kernel_spmd(nc, in_maps, list(range(NCORES)), trace=trace,
                               **spmd_kwargs)
    return gather_out(res.results), res


def kernel(x, w_q, w_k, w_v, w_o):
    out, _ = run(x, w_q, w_k, w_v, w_o)
    return out
